# revision 24
# baseline (speedup 1.0000x reference)
"""Distributed causal self-attention (B=2, T=2048, C=1024, H=16, hs=64) on 8 TRN2 NeuronCores.

Sharding (Megatron-style per the hint): core c handles batch b=c//4 and head
group g=c%4 (4 heads).  Per core:
  - QKV projection for its 4 heads only (column-parallel c_attn),
  - RoPE folded into host-side weight-row permutations (main/aux weight pairs
    combined on-device as slab = ps_m*T1 + ps_a*T2),
  - causal attention with scores kept transposed [keys, queries]; softmax
    denominator from a ones-column appended to V,
  - row-parallel c_proj partials, chunked ReduceScatter over the 4 cores of
    the batch; rank r ends with output channels [256r:256r+256].

v2 schedule (vs baseline): single merged Tile scope; QKV-projection work for
T-chunk tcn+1 is interleaved into the attention rounds of query-chunk tcn so
the PE never idles during the exp-bound attention stretches; scores run one
pr-pair ahead of A@V in the PE queue; softmax tails (reciprocal/broadcast/
normalize) are emitted one head late so their cross-engine waits never
head-block the Vector/GpSimd queues; reciprocal uses the fast approx op;
rope-combine adds run on GpSimd; final output hops go DRAM->DRAM on the
Scalar DMA queue so a ReduceScatter-gated DMA never plugs the Sync rings.
"""

import sys

sys.path.insert(0, "/opt/trn_rl_repo")

from collections import deque

import numpy as np

from concourse import bacc, tile, mybir
from concourse.bass_utils import run_bass_kernel_spmd

F32 = mybir.dt.float32
F32R = mybir.dt.float32r
BF16 = mybir.dt.bfloat16

B, T, C, H, HS = 2, 2048, 1024, 16, 64
HALF = HS // 2  # 32
N_CORES = 8
QCHUNK = 512
NQC = T // QCHUNK  # 4
KBLK = 128
NKB = T // KBLK  # 16
N_CB = C // 128  # 8
RG = [[0, 1, 2, 3], [4, 5, 6, 7]]


# ----------------------------------------------------------------------------
# Host-side constant prep (identical to baseline)
# ----------------------------------------------------------------------------

def _rope_tables():
    """T1/T2 (128, T): rope as new = main*T1 + aux*T2, lane-aligned."""
    pos = np.arange(T, dtype=np.float64)
    freq = 1.0 / (10000.0 ** (np.arange(0, HS, 2, dtype=np.float64) / HS))
    ang = pos[:, None] * freq[None, :]
    c, s = np.cos(ang), np.sin(ang)
    cp = np.roll(c, -1, axis=1)
    sp = np.roll(s, -1, axis=1)
    A = c - s * sp
    Bt = s * cp
    T1 = np.empty((128, T), dtype=np.float32)
    T2 = np.empty((128, T), dtype=np.float32)
    for hh in range(2):
        T1[64 * hh : 64 * hh + 32] = c.T
        T1[64 * hh + 32 : 64 * hh + 64] = A.T
        T2[64 * hh : 64 * hh + 32] = (-s).T
        T2[64 * hh + 32 : 64 * hh + 64] = Bt.T
    return T1, T2


def _qk_weights(w_attn, g):
    """wqk_host (512, 2048) for head group g (main|aux column pairs per cb)."""
    cols = []
    for qk in range(2):
        for sl in range(2):
            main = np.empty((128, C), dtype=np.float32)
            aux = np.empty((128, C), dtype=np.float32)
            for hh in range(2):
                h_glob = 4 * g + 2 * sl + hh
                base = qk * C + 64 * h_glob
                for i in range(HALF):
                    main[hh * 64 + i] = w_attn[base + 2 * i]
                    main[hh * 64 + 32 + i] = w_attn[base + 2 * i + 1]
                    aux[hh * 64 + i] = w_attn[base + 2 * ((i - 1) % HALF) + 1]
                    aux[hh * 64 + 32 + i] = w_attn[base + 2 * ((i + 1) % HALF)]
            cols.append(main)
            cols.append(aux)
    wqkT = np.concatenate([blk.T for blk in cols], axis=1)  # (C, 1024)
    return np.ascontiguousarray(
        wqkT.reshape(8, 128, 4, 256).transpose(2, 1, 0, 3).reshape(512, 2048)
    )


def _v_weights(w_attn, g):
    wv = np.empty((C, 256), dtype=np.float32)
    for j in range(4):
        h_glob = 4 * g + j
        wv[:, 64 * j : 64 * j + 64] = w_attn[2 * C + 64 * h_glob : 2 * C + 64 * h_glob + 64].T
    return np.ascontiguousarray(wv)


def _proj_weights(w_proj, g):
    wp = np.empty((256, C), dtype=np.float32)
    for cb in range(2):
        for jj in range(2):
            h_glob = 4 * g + 2 * cb + jj
            blk = w_proj[:, 64 * h_glob : 64 * h_glob + 64].T  # (64, 1024)
            wp[128 * cb + 64 * jj : 128 * cb + 64 * jj + 64] = blk
    return np.ascontiguousarray(wp)


def _mask_tiles():
    """(4*128, 1024) f32: mask_j duplicated along cols so one tensor_tensor
    masks both heads' halves of a paired et tile."""
    m = np.zeros((4, 128, 2 * QCHUNK), dtype=np.float32)
    q = np.arange(QCHUNK)[None, :]
    k = np.arange(128)[:, None]
    for j in range(4):
        base = (q >= 128 * j + k).astype(np.float32)
        m[j][:, 0:QCHUNK] = base
        m[j][:, QCHUNK:] = base
    return np.ascontiguousarray(m.reshape(4 * 128, 2 * QCHUNK))


def _bf16(a):
    import ml_dtypes
    return np.asarray(a, dtype=np.float32).astype(ml_dtypes.bfloat16)


def prepare_in_maps(x, w_attn, w_proj):
    x = np.asarray(x, dtype=np.float32)
    w_attn = np.asarray(w_attn, dtype=np.float32)
    w_proj = np.asarray(w_proj, dtype=np.float32)
    T1, T2 = _rope_tables()
    xh = {}
    for b in range(B):
        xT = np.ascontiguousarray(x[b].T)  # (C, T)
        xh[b] = np.ascontiguousarray(
            xT.reshape(C, NQC, QCHUNK).transpose(1, 0, 2).reshape(NQC * C, QCHUNK)
        )
    in_maps = []
    for core in range(N_CORES):
        b, g = core // 4, core % 4
        in_maps.append(
            {
                "xh": _bf16(xh[b]),
                "wqk": _bf16(_qk_weights(w_attn, g)),
                "wv": _bf16(_v_weights(w_attn, g)),
                "t1": T1,
                "t2": T2,
                "wproj": _bf16(_proj_weights(w_proj, g)),
                "onesv": _bf16(np.ones((128, 4 * NKB), dtype=np.float32)),
                "masks": _bf16(_mask_tiles()),
            }
        )
    return in_maps


# ----------------------------------------------------------------------------
# Device kernel
# ----------------------------------------------------------------------------

DBG = False


def build_nc(seq=T, debug=False):
    T_, NQC_, NKB_ = seq, seq // QCHUNK, seq // KBLK
    nc = bacc.Bacc("TRN2", target_bir_lowering=False, debug=debug, num_devices=N_CORES)

    xh = nc.dram_tensor("xh", [NQC_ * C, QCHUNK], BF16, kind="ExternalInput").ap()
    wqk = nc.dram_tensor("wqk", [512, 2048], BF16, kind="ExternalInput").ap()
    wv = nc.dram_tensor("wv", [C, 256], BF16, kind="ExternalInput").ap()
    t1 = nc.dram_tensor("t1", [128, T_], F32, kind="ExternalInput").ap()
    t2 = nc.dram_tensor("t2", [128, T_], F32, kind="ExternalInput").ap()
    wproj = nc.dram_tensor("wproj", [256, C], BF16, kind="ExternalInput").ap()
    onesv = nc.dram_tensor("onesv", [128, 4 * NKB_], BF16, kind="ExternalInput").ap()
    masks = nc.dram_tensor("masks", [4 * 128, 2 * QCHUNK], BF16, kind="ExternalInput").ap()
    out = nc.dram_tensor("out", [256, T_], BF16, kind="ExternalOutput").ap()
    if DBG:
        dbg_slab = nc.dram_tensor("dbg_slab", [128, T_], BF16, kind="ExternalOutput").ap()
        dbg_vsl = nc.dram_tensor("dbg_vsl", [128, 4 * (T_ // KBLK) * 65], BF16, kind="ExternalOutput").ap()
        dbg_ysl = nc.dram_tensor("dbg_ysl", [128, T_], BF16, kind="ExternalOutput").ap()
        dbg_recip = nc.dram_tensor("dbg_recip", [16, QCHUNK], F32, kind="ExternalOutput").ap()
        dbg_rso = nc.dram_tensor("dbg_rso", [256, QCHUNK], BF16, kind="ExternalOutput").ap()

    mult = mybir.AluOpType.mult
    add = mybir.AluOpType.add

    with tile.TileContext(nc) as tc:
        with (
            tc.tile_pool(name="persist", bufs=1) as persist,
            tc.tile_pool(name="dramp", bufs=1, space="DRAM") as dramp,
            tc.tile_pool(name="pa", bufs=2) as pa,            # xtc tiles
            tc.tile_pool(name="pa_tmp", bufs=4) as pa_tmp,    # rope tmp f32
            tc.tile_pool(name="pb", bufs=5) as pb,            # et tiles
            tc.tile_pool(name="pb2", bufs=2) as pb2,          # recip / bcast
            tc.tile_pool(name="pc_o", bufs=4) as pc_o,        # proj out bf16
            tc.tile_pool(name="psW", bufs=2, space="PSUM") as psW,  # shared [128,512]
            tc.tile_pool(name="psS", bufs=2, space="PSUM") as psS,  # scores [128,1024]
            tc.tile_pool(name="psY", bufs=2, space="PSUM") as psY,  # y accum [65,512]
        ):
            t1s = persist.tile([128, T_], F32, name="t1s")
            t2s = persist.tile([128, T_], F32, name="t2s")
            slabs = [persist.tile([128, T_], BF16, name=f"slab{s}") for s in range(4)]
            vslab = persist.tile([128, 4 * NKB_ * 65], BF16, name="vslab")
            vs4 = vslab[:].rearrange("p (h k d) -> p h k d", h=4, k=NKB_, d=65)
            yslabs = [persist.tile([128, T_], BF16, name=f"yslab{u}") for u in range(2)]
            wqs = [persist.tile([128, 2048], BF16, name=f"wqs{s}") for s in range(4)]
            wvs = persist.tile([128, 8 * 256], BF16, name="wvs")
            mks = persist.tile([128, 4 * 2 * QCHUNK], BF16, name="mks")
            wps = persist.tile([128, 2 * C], BF16, name="wps")
            ones64 = persist.tile([1, 64], BF16, name="ones64")
            nc.vector.memset(ones64[:], 1.0)

            rsin = [dramp.tile([C, QCHUNK], BF16, name=f"rsin{q}") for q in range(NQC_)]
            rsout = [dramp.tile([256, QCHUNK], BF16, name=f"rsout{q}") for q in range(NQC_)]

            # ---------------- preloads ----------------
            # sync queue: what phase A needs first
            nc.sync.dma_start(out=wqs[0][:], in_=wqk[0:128, :])
            xtcs = {}

            def load_xtc(tcn):
                t = pa.tile([128, 8 * QCHUNK], BF16, name="xtc", tag="xtc")
                for cb in range(N_CB):
                    nc.sync.dma_start(
                        out=t[:, QCHUNK * cb : QCHUNK * (cb + 1)],
                        in_=xh[C * tcn + 128 * cb : C * tcn + 128 * (cb + 1), :],
                    )
                xtcs[tcn] = t

            load_xtc(0)
            for s in range(1, 4):
                nc.sync.dma_start(out=wqs[s][:], in_=wqk[128 * s : 128 * s + 128, :])
            # scalar queue: constants needed a bit later
            nc.scalar.dma_start(out=t1s[:], in_=t1)
            nc.scalar.dma_start(out=t2s[:], in_=t2)
            nc.scalar.dma_start(
                out=wvs[:].rearrange("p (c w) -> p c w", c=8),
                in_=wv.rearrange("(c p) w -> p c w", p=128),
            )
            nc.scalar.dma_start(
                out=vs4[:, :, :, 64:65],
                in_=onesv.rearrange("p (h k w) -> p h k w", h=4, k=NKB_, w=1),
            )
            nc.scalar.dma_start(
                out=mks[:].rearrange("p (j w) -> p j w", j=4),
                in_=masks.rearrange("(j p) w -> p j w", p=128),
            )
            nc.scalar.dma_start(
                out=wps[:].rearrange("p (c w) -> p c w", c=2),
                in_=wproj.rearrange("(c p) w -> p c w", p=128),
            )
            # dummy exp: pulls the ~2.7us ACT table load into the phase-A window
            warm = pb2.tile([1, 8], F32, name="warm", tag="den")
            nc.scalar.activation(
                warm[:], t1s[0:1, 0:8], mybir.ActivationFunctionType.Exp, scale=0.0
            )

            # ---------------- phase-A work chains ----------------
            def chain_prefetch(tcn):
                def go():
                    load_xtc(tcn)
                return go, 0

            def chain_m(tcn, s):
                def go():
                    xtc = xtcs[tcn]
                    tcol = slice(QCHUNK * tcn, QCHUNK * (tcn + 1))
                    ps_m = psW.tile([128, QCHUNK], F32, name="ps_m", tag="w")
                    for cb in range(N_CB):
                        nc.tensor.matmul(
                            ps_m[:],
                            lhsT=wqs[s][:, 256 * cb : 256 * cb + 128],
                            rhs=xtc[:, QCHUNK * cb : QCHUNK * (cb + 1)],
                            start=(cb == 0), stop=(cb == N_CB - 1),
                        )
                    tmp1 = pa_tmp.tile([128, QCHUNK], F32, name="tmp1", tag="tmp")
                    nc.vector.tensor_tensor(tmp1[:], ps_m[:], t1s[:, tcol], mult)
                    return tmp1
                return go, 1730

            def chain_a(tcn, s, get_tmp1):
                def go():
                    xtc = xtcs[tcn]
                    tcol = slice(QCHUNK * tcn, QCHUNK * (tcn + 1))
                    ps_a = psW.tile([128, QCHUNK], F32, name="ps_a", tag="w")
                    for cb in range(N_CB):
                        nc.tensor.matmul(
                            ps_a[:],
                            lhsT=wqs[s][:, 256 * cb + 128 : 256 * cb + 256],
                            rhs=xtc[:, QCHUNK * cb : QCHUNK * (cb + 1)],
                            start=(cb == 0), stop=(cb == N_CB - 1),
                        )
                    tmp2 = pa_tmp.tile([128, QCHUNK], F32, name="tmp2", tag="tmp")
                    nc.vector.tensor_tensor(tmp2[:], ps_a[:], t2s[:, tcol], mult)
                    tmp1 = get_tmp1()
                    # NOT gpsimd: mixing tensor_tensor with partition_broadcast
                    # on GpSimd thrashes the Q7 library (~6us reload per switch)
                    nc.vector.tensor_tensor(slabs[s][:, tcol], tmp1[:], tmp2[:], add)
                return go, 1730

            def chain_v(tcn, tb):
                def go():
                    xtc = xtcs[tcn]
                    kb = 4 * tcn + tb
                    psv = psW.tile([128, 256], F32, name="psv", tag="w")
                    for cb in range(N_CB):
                        nc.tensor.matmul(
                            psv[:],
                            lhsT=xtc[:, QCHUNK * cb + 128 * tb : QCHUNK * cb + 128 * (tb + 1)],
                            rhs=wvs[:, 256 * cb : 256 * (cb + 1)],
                            start=(cb == 0), stop=(cb == N_CB - 1),
                        )
                    # Scalar engine is idle during phase A; PSUM->SBUF copy is
                    # cheap there and keeps the Vector queue free for rope math
                    nc.scalar.copy(
                        vs4[:, :, kb, 0:64], psv[:].rearrange("p (h d) -> p h d", h=4)
                    )
                return go, 880

            def make_a_chains(tcn):
                chains = []
                if tcn + 1 < NQC_:
                    chains.append((tcn, chain_prefetch(tcn + 1)))
                for s in range(4):
                    holder = {}
                    m_go, m_ns = chain_m(tcn, s)

                    def m_wrap(m_go=m_go, holder=holder):
                        holder["tmp1"] = m_go()
                    chains.append((tcn, (m_wrap, m_ns)))
                    a_go, a_ns = chain_a(tcn, s, lambda holder=holder: holder["tmp1"])
                    chains.append((tcn, (a_go, a_ns)))
                for tb in range(4):
                    chains.append((tcn, chain_v(tcn, tb)))
                return chains

            awork = deque()
            for tcn in range(1, NQC_):
                awork.extend(make_a_chains(tcn))

            # phase A for tcn=0: dense
            for _, (go, _ns) in make_a_chains(0):
                go()

            state = {"deficit": 0.0}

            def pace(max_tcn, budget=True):
                while awork and awork[0][0] <= max_tcn:
                    _, (go, cost) = awork[0]
                    if budget and state["deficit"] < cost:
                        break
                    awork.popleft()
                    go()
                    state["deficit"] -= cost

            # ---------------- attention + proj + RS ----------------
            # Heads are processed in (u=0, u=1) pairs: their score matmuls have
            # K=64 contractions on row groups h0/h64, so alternating them lets
            # the PE overlap the two matmuls AND pull LDWEIGHTS ahead.
            ROUND_GAP = 450.0
            pending_tail = []  # deferred softmax tails (ycp-based)

            def emit_tails():
                while pending_tail:
                    ycp, hp, u, qcol, r = pending_tail.pop(0)
                    # stage denom at partition 0 (custom DVE op requires it)
                    den = pb2.tile([1, QCHUNK], F32, name="den", tag="den")
                    nc.vector.tensor_copy(den[:], ycp[64:65, :])
                    recip = pb2.tile([1, QCHUNK], F32, name="recip", tag="recip")
                    nc.vector.reciprocal_approx_fast(out=recip[:], in_=den[:])
                    recb = pb2.tile([1, QCHUNK], BF16, name="recb", tag="recb")
                    nc.vector.tensor_copy(recb[:], recip[:])
                    # broadcast across partitions with a K=1 matmul
                    bc = psW.tile([64, QCHUNK], F32, name="bc", tag="w")
                    nc.tensor.matmul(
                        bc[:], lhsT=ones64[:], rhs=recb[:], start=True, stop=True,
                    )
                    nc.vector.tensor_tensor(
                        yslabs[hp][64 * u : 64 * u + 64, qcol],
                        ycp[0:64, :], bc[:], mult,
                    )
                    if DBG:
                        nc.sync.dma_start(out=dbg_recip[r : r + 1, :], in_=recip[:])

            for qc in range(NQC_):
                qcol = slice(QCHUNK * qc, QCHUNK * (qc + 1))
                nblocks = 4 * qc + 4
                for hp in range(2):
                    qsl = slabs[hp]
                    ksl = slabs[2 + hp]
                    j0, j1 = 2 * hp, 2 * hp + 1
                    yps0 = psY.tile([65, QCHUNK], F32, name="yps0", tag="yps")
                    yps1 = psY.tile([65, QCHUNK], F32, name="yps1", tag="yps")
                    ets = {}

                    def av_pair(kb, yps0=yps0, yps1=yps1, j0=j0, j1=j1, ets=ets, nblocks=nblocks):
                        etp = ets.pop(kb)
                        nc.tensor.matmul(
                            yps0[:],
                            lhsT=vslab[:, (j0 * NKB_ + kb) * 65 : (j0 * NKB_ + kb + 1) * 65],
                            rhs=etp[:, 0:QCHUNK],
                            start=(kb == 0), stop=(kb == nblocks - 1),
                        )
                        nc.tensor.matmul(
                            yps1[:],
                            lhsT=vslab[:, (j1 * NKB_ + kb) * 65 : (j1 * NKB_ + kb + 1) * 65],
                            rhs=etp[:, QCHUNK : 2 * QCHUNK],
                            start=(kb == 0), stop=(kb == nblocks - 1),
                        )

                    for kb in range(nblocks):
                        kcol = slice(128 * kb, 128 * (kb + 1))
                        sp = psS.tile([128, 1024], F32, name="sp", tag="sp")
                        nc.tensor.matmul(
                            sp[:, 0:QCHUNK],
                            lhsT=ksl[0:64, kcol], rhs=qsl[0:64, qcol],
                            start=True, stop=True,
                        )
                        nc.tensor.matmul(
                            sp[:, QCHUNK : 2 * QCHUNK],
                            lhsT=ksl[64:128, kcol], rhs=qsl[64:128, qcol],
                            start=True, stop=True,
                        )
                        et = pb.tile([128, 1024], BF16, name="et", tag="et")
                        nc.scalar.activation(
                            et[:], sp[:], mybir.ActivationFunctionType.Exp,
                            scale=0.125,
                        )
                        if kb >= 4 * qc:  # block straddles the causal diagonal
                            jd = kb - 4 * qc
                            nc.gpsimd.tensor_tensor(
                                et[:], et[:],
                                mks[:, 1024 * jd : 1024 * (jd + 1)], mult,
                            )
                        ets[kb] = et
                        if kb == 1:
                            emit_tails()
                        # interleave ready phase-A work into the exp-bound gap
                        state["deficit"] += ROUND_GAP
                        pace(qc + 1)
                        if kb >= 1:
                            av_pair(kb - 1)
                    av_pair(nblocks - 1)
                    # free the yps PSUM slots quickly; normalization is deferred
                    ycp0 = pb2.tile([65, QCHUNK], F32, name="ycp0", tag="ycp", bufs=4)
                    nc.vector.tensor_copy(ycp0[:], yps0[:])
                    ycp1 = pb2.tile([65, QCHUNK], F32, name="ycp1", tag="ycp", bufs=4)
                    nc.vector.tensor_copy(ycp1[:], yps1[:])
                    pending_tail.append((ycp0, hp, 0, qcol, 4 * qc + j0))
                    pending_tail.append((ycp1, hp, 1, qcol, 4 * qc + j1))

                # drain remaining A-work for the next T-chunk, tails overlap it
                state["deficit"] += 2 * 1730
                pace(qc + 1)
                emit_tails()
                pace(qc + 1, budget=False)

                # proj partials for this T-chunk, then ReduceScatter
                for e in range(8):
                    pso = psW.tile([128, QCHUNK], F32, name="pso", tag="w")
                    for cb in range(2):
                        nc.tensor.matmul(
                            pso[:],
                            lhsT=wps[:, C * cb + 128 * e : C * cb + 128 * (e + 1)],
                            rhs=yslabs[cb][:, qcol],
                            start=(cb == 0), stop=(cb == 1),
                        )
                    osb = pc_o.tile([128, QCHUNK], BF16, name="osb", tag="osb")
                    nc.vector.tensor_copy(osb[:], pso[:])
                    nc.sync.dma_start(out=rsin[qc][128 * e : 128 * (e + 1), :], in_=osb[:])
                nc.gpsimd.collective_compute(
                    "ReduceScatter", add, replica_groups=RG,
                    ins=[rsin[qc][:].opt()], outs=[rsout[qc][:].opt()],
                )

            # output hops: DRAM->DRAM on the Scalar DMA queue. tile_wait_until
            # pins them to the END of the scalar queue in the scheduler's model
            # (they wait on RS completion; scheduled early they head-block the
            # engine queue and freeze the exp stream for tens of us).
            for qc in range(NQC_):
                qcol = slice(QCHUNK * qc, QCHUNK * (qc + 1))
                with tc.tile_wait_until(1.0 + 0.1 * qc):
                    nc.scalar.dma_start(out=out[:, qcol], in_=rsout[qc][:, :])
            if DBG:
                nc.sync.dma_start(out=dbg_slab, in_=slabs[0][:])
                nc.sync.dma_start(out=dbg_vsl, in_=vslab[:])
                nc.sync.dma_start(out=dbg_ysl, in_=yslabs[0][:])
                nc.sync.dma_start(out=dbg_rso, in_=rsout[0][:, :])

    nc.compile()
    return nc


_NC_CACHE = {}


def get_nc():
    if "nc" not in _NC_CACHE:
        _NC_CACHE["nc"] = build_nc()
    return _NC_CACHE["nc"]


def assemble(results):
    out = np.empty((B, T, C), dtype=np.float32)
    for core in range(N_CORES):
        b, r = core // 4, core % 4
        out[b, :, 256 * r : 256 * (r + 1)] = np.asarray(results[core]["out"], dtype=np.float32).T
    return out


def kernel(x, w_attn, w_proj):
    in_maps = prepare_in_maps(x, w_attn, w_proj)
    nc = get_nc()
    res = run_bass_kernel_spmd(nc, in_maps, core_ids=list(range(N_CORES)))
    return assemble(res.results)


# revision 29
# speedup vs baseline: 1.1736x; 1.1736x over previous
"""Distributed causal self-attention (B=2, T=2048, C=1024, H=16, hs=64) on 8 TRN2 NeuronCores.

Sharding (Megatron-style per the hint): core c handles batch b=c//4 and head
group g=c%4 (4 heads).  Per core:
  - QKV projection for its 4 heads only (column-parallel c_attn),
  - RoPE folded into host-side weight-row permutations (main/aux weight pairs
    combined on-device as slab = ps_m*T1 + ps_a*T2),
  - causal attention with scores kept transposed [keys, queries]; softmax
    denominator from a ones-column appended to V,
  - row-parallel c_proj partials, chunked ReduceScatter over the 4 cores of
    the batch; rank r ends with output channels [256r:256r+256].

v2 schedule (vs baseline): single merged Tile scope; QKV-projection work for
T-chunk tcn+1 is interleaved into the attention rounds of query-chunk tcn so
the PE never idles during the exp-bound attention stretches; scores run one
pr-pair ahead of A@V in the PE queue; softmax tails (reciprocal/broadcast/
normalize) are emitted one head late so their cross-engine waits never
head-block the Vector/GpSimd queues; reciprocal uses the fast approx op;
rope-combine adds run on GpSimd; final output hops go DRAM->DRAM on the
Scalar DMA queue so a ReduceScatter-gated DMA never plugs the Sync rings.
"""

import sys

sys.path.insert(0, "/opt/trn_rl_repo")

from collections import deque

import numpy as np

from concourse import bacc, tile, mybir
from concourse.bass_utils import run_bass_kernel_spmd

F32 = mybir.dt.float32
F32R = mybir.dt.float32r
BF16 = mybir.dt.bfloat16

B, T, C, H, HS = 2, 2048, 1024, 16, 64
HALF = HS // 2  # 32
N_CORES = 8
QCHUNK = 512
NQC = T // QCHUNK  # 4
KBLK = 128
NKB = T // KBLK  # 16
N_CB = C // 128  # 8
RG = [[0, 1, 2, 3], [4, 5, 6, 7]]


# ----------------------------------------------------------------------------
# Host-side constant prep (identical to baseline)
# ----------------------------------------------------------------------------

def _rope_tables():
    """T1/T2 (128, T): rope as new = main*T1 + aux*T2, lane-aligned."""
    pos = np.arange(T, dtype=np.float64)
    freq = 1.0 / (10000.0 ** (np.arange(0, HS, 2, dtype=np.float64) / HS))
    ang = pos[:, None] * freq[None, :]
    c, s = np.cos(ang), np.sin(ang)
    cp = np.roll(c, -1, axis=1)
    sp = np.roll(s, -1, axis=1)
    A = c - s * sp
    Bt = s * cp
    T1 = np.empty((128, T), dtype=np.float32)
    T2 = np.empty((128, T), dtype=np.float32)
    for hh in range(2):
        T1[64 * hh : 64 * hh + 32] = c.T
        T1[64 * hh + 32 : 64 * hh + 64] = A.T
        T2[64 * hh : 64 * hh + 32] = (-s).T
        T2[64 * hh + 32 : 64 * hh + 64] = Bt.T
    return T1, T2


def _qk_weights(w_attn, g):
    """wqk_host (512, 2048) for head group g (main|aux column pairs per cb)."""
    cols = []
    for qk in range(2):
        for sl in range(2):
            main = np.empty((128, C), dtype=np.float32)
            aux = np.empty((128, C), dtype=np.float32)
            for hh in range(2):
                h_glob = 4 * g + 2 * sl + hh
                base = qk * C + 64 * h_glob
                for i in range(HALF):
                    main[hh * 64 + i] = w_attn[base + 2 * i]
                    main[hh * 64 + 32 + i] = w_attn[base + 2 * i + 1]
                    aux[hh * 64 + i] = w_attn[base + 2 * ((i - 1) % HALF) + 1]
                    aux[hh * 64 + 32 + i] = w_attn[base + 2 * ((i + 1) % HALF)]
            cols.append(main)
            cols.append(aux)
    wqkT = np.concatenate([blk.T for blk in cols], axis=1)  # (C, 1024)
    return np.ascontiguousarray(
        wqkT.reshape(8, 128, 4, 256).transpose(2, 1, 0, 3).reshape(512, 2048)
    )


def _v_weights(w_attn, g):
    wv = np.empty((C, 256), dtype=np.float32)
    for j in range(4):
        h_glob = 4 * g + j
        wv[:, 64 * j : 64 * j + 64] = w_attn[2 * C + 64 * h_glob : 2 * C + 64 * h_glob + 64].T
    return np.ascontiguousarray(wv)


def _proj_weights(w_proj, g):
    wp = np.empty((256, C), dtype=np.float32)
    for cb in range(2):
        for jj in range(2):
            h_glob = 4 * g + 2 * cb + jj
            blk = w_proj[:, 64 * h_glob : 64 * h_glob + 64].T  # (64, 1024)
            wp[128 * cb + 64 * jj : 128 * cb + 64 * jj + 64] = blk
    return np.ascontiguousarray(wp)


def _mask_tiles():
    """(4*128, 512) f32: mask_j[k, q] = 1 if q >= 128*j + k else 0, j=0..3."""
    m = np.zeros((4, 128, QCHUNK), dtype=np.float32)
    q = np.arange(QCHUNK)[None, :]
    k = np.arange(128)[:, None]
    for j in range(4):
        m[j] = (q >= 128 * j + k).astype(np.float32)
    return np.ascontiguousarray(m.reshape(4 * 128, QCHUNK))


def _bf16(a):
    import ml_dtypes
    return np.asarray(a, dtype=np.float32).astype(ml_dtypes.bfloat16)


def prepare_in_maps(x, w_attn, w_proj):
    x = np.asarray(x, dtype=np.float32)
    w_attn = np.asarray(w_attn, dtype=np.float32)
    w_proj = np.asarray(w_proj, dtype=np.float32)
    T1, T2 = _rope_tables()
    xh = {}
    for b in range(B):
        xT = np.ascontiguousarray(x[b].T)  # (C, T)
        xh[b] = np.ascontiguousarray(
            xT.reshape(C, NQC, QCHUNK).transpose(1, 0, 2).reshape(NQC * C, QCHUNK)
        )
    in_maps = []
    for core in range(N_CORES):
        b, g = core // 4, core % 4
        in_maps.append(
            {
                "xh": _bf16(xh[b]),
                "wqk": _bf16(_qk_weights(w_attn, g)),
                "wv": _bf16(_v_weights(w_attn, g)),
                "t1": T1,
                "t2": T2,
                "wproj": _bf16(_proj_weights(w_proj, g)),
                "onesv": _bf16(np.ones((128, 4 * NKB), dtype=np.float32)),
                "masks": _bf16(_mask_tiles()),
            }
        )
    return in_maps


# ----------------------------------------------------------------------------
# Device kernel
# ----------------------------------------------------------------------------

DBG = False


def build_nc(seq=T, debug=False):
    T_, NQC_, NKB_ = seq, seq // QCHUNK, seq // KBLK
    nc = bacc.Bacc("TRN2", target_bir_lowering=False, debug=debug, num_devices=N_CORES)

    xh = nc.dram_tensor("xh", [NQC_ * C, QCHUNK], BF16, kind="ExternalInput").ap()
    wqk = nc.dram_tensor("wqk", [512, 2048], BF16, kind="ExternalInput").ap()
    wv = nc.dram_tensor("wv", [C, 256], BF16, kind="ExternalInput").ap()
    t1 = nc.dram_tensor("t1", [128, T_], F32, kind="ExternalInput").ap()
    t2 = nc.dram_tensor("t2", [128, T_], F32, kind="ExternalInput").ap()
    wproj = nc.dram_tensor("wproj", [256, C], BF16, kind="ExternalInput").ap()
    onesv = nc.dram_tensor("onesv", [128, 4 * NKB_], BF16, kind="ExternalInput").ap()
    masks = nc.dram_tensor("masks", [4 * 128, QCHUNK], BF16, kind="ExternalInput").ap()
    out = nc.dram_tensor("out", [256, T_], BF16, kind="ExternalOutput").ap()
    if DBG:
        dbg_slab = nc.dram_tensor("dbg_slab", [128, T_], BF16, kind="ExternalOutput").ap()
        dbg_vsl = nc.dram_tensor("dbg_vsl", [128, 4 * (T_ // KBLK) * 65], BF16, kind="ExternalOutput").ap()
        dbg_ysl = nc.dram_tensor("dbg_ysl", [128, T_], BF16, kind="ExternalOutput").ap()
        dbg_recip = nc.dram_tensor("dbg_recip", [16, QCHUNK], F32, kind="ExternalOutput").ap()
        dbg_rso = nc.dram_tensor("dbg_rso", [256, QCHUNK], BF16, kind="ExternalOutput").ap()

    mult = mybir.AluOpType.mult
    add = mybir.AluOpType.add

    with tile.TileContext(nc) as tc:
        with (
            tc.tile_pool(name="persist", bufs=1) as persist,
            tc.tile_pool(name="dramp", bufs=1, space="DRAM") as dramp,
            tc.tile_pool(name="pa", bufs=2) as pa,            # xtc tiles
            tc.tile_pool(name="pa_tmp", bufs=4) as pa_tmp,    # rope tmp f32
            tc.tile_pool(name="pb", bufs=5) as pb,            # et tiles
            tc.tile_pool(name="pb2", bufs=2) as pb2,          # recip / bcast
            tc.tile_pool(name="pc_o", bufs=4) as pc_o,        # proj out bf16
            tc.tile_pool(name="psW", bufs=2, space="PSUM") as psW,  # shared [128,512]
            tc.tile_pool(name="psS", bufs=2, space="PSUM") as psS,  # scores [128,1024]
            tc.tile_pool(name="psY", bufs=2, space="PSUM") as psY,  # y accum [65,512]
        ):
            t1s = persist.tile([128, T_], F32, name="t1s")
            t2s = persist.tile([128, T_], F32, name="t2s")
            slabs = [persist.tile([128, T_], BF16, name=f"slab{s}") for s in range(4)]
            vslab = persist.tile([128, 4 * NKB_ * 65], BF16, name="vslab")
            vs4 = vslab[:].rearrange("p (h k d) -> p h k d", h=4, k=NKB_, d=65)
            yslabs = [persist.tile([128, T_], BF16, name=f"yslab{u}") for u in range(2)]
            wqs = [persist.tile([128, 2048], BF16, name=f"wqs{s}") for s in range(4)]
            wvs = persist.tile([128, 8 * 256], BF16, name="wvs")
            mks = persist.tile([128, 4 * QCHUNK], BF16, name="mks")
            wps = persist.tile([128, 2 * C], BF16, name="wps")

            rsin = [dramp.tile([C, QCHUNK], BF16, name=f"rsin{q}") for q in range(NQC_)]
            rsout = [dramp.tile([256, QCHUNK], BF16, name=f"rsout{q}") for q in range(NQC_)]

            # ---------------- preloads ----------------
            # sync queue: what phase A needs first
            nc.sync.dma_start(out=wqs[0][:], in_=wqk[0:128, :])
            xtcs = {}

            def load_xtc(tcn):
                t = pa.tile([128, 8 * QCHUNK], BF16, name="xtc", tag="xtc")
                for cb in range(N_CB):
                    nc.sync.dma_start(
                        out=t[:, QCHUNK * cb : QCHUNK * (cb + 1)],
                        in_=xh[C * tcn + 128 * cb : C * tcn + 128 * (cb + 1), :],
                    )
                xtcs[tcn] = t

            load_xtc(0)
            for s in range(1, 4):
                nc.sync.dma_start(out=wqs[s][:], in_=wqk[128 * s : 128 * s + 128, :])
            # scalar queue: constants needed a bit later
            nc.scalar.dma_start(out=t1s[:], in_=t1)
            nc.scalar.dma_start(out=t2s[:], in_=t2)
            nc.scalar.dma_start(
                out=wvs[:].rearrange("p (c w) -> p c w", c=8),
                in_=wv.rearrange("(c p) w -> p c w", p=128),
            )
            nc.scalar.dma_start(
                out=vs4[:, :, :, 64:65],
                in_=onesv.rearrange("p (h k w) -> p h k w", h=4, k=NKB_, w=1),
            )
            nc.scalar.dma_start(
                out=mks[:].rearrange("p (j w) -> p j w", j=4),
                in_=masks.rearrange("(j p) w -> p j w", p=128),
            )
            nc.scalar.dma_start(
                out=wps[:].rearrange("p (c w) -> p c w", c=2),
                in_=wproj.rearrange("(c p) w -> p c w", p=128),
            )
            # dummy exp: pulls the ~2.7us ACT table load into the phase-A window
            warm = pb2.tile([1, 8], F32, name="warm", tag="den")
            nc.scalar.activation(
                warm[:], t1s[0:1, 0:8], mybir.ActivationFunctionType.Exp, scale=0.0
            )

            # ---------------- phase-A work chains ----------------
            def chain_prefetch(tcn):
                def go():
                    load_xtc(tcn)
                return go, 0

            def chain_m(tcn, s):
                def go():
                    xtc = xtcs[tcn]
                    tcol = slice(QCHUNK * tcn, QCHUNK * (tcn + 1))
                    ps_m = psW.tile([128, QCHUNK], F32, name="ps_m", tag="w")
                    for cb in range(N_CB):
                        nc.tensor.matmul(
                            ps_m[:],
                            lhsT=wqs[s][:, 256 * cb : 256 * cb + 128],
                            rhs=xtc[:, QCHUNK * cb : QCHUNK * (cb + 1)],
                            start=(cb == 0), stop=(cb == N_CB - 1),
                        )
                    tmp1 = pa_tmp.tile([128, QCHUNK], F32, name="tmp1", tag="tmp")
                    nc.vector.tensor_tensor(tmp1[:], ps_m[:], t1s[:, tcol], mult)
                    return tmp1
                return go, 1730

            def chain_a(tcn, s, get_tmp1):
                def go():
                    xtc = xtcs[tcn]
                    tcol = slice(QCHUNK * tcn, QCHUNK * (tcn + 1))
                    ps_a = psW.tile([128, QCHUNK], F32, name="ps_a", tag="w")
                    for cb in range(N_CB):
                        nc.tensor.matmul(
                            ps_a[:],
                            lhsT=wqs[s][:, 256 * cb + 128 : 256 * cb + 256],
                            rhs=xtc[:, QCHUNK * cb : QCHUNK * (cb + 1)],
                            start=(cb == 0), stop=(cb == N_CB - 1),
                        )
                    tmp2 = pa_tmp.tile([128, QCHUNK], F32, name="tmp2", tag="tmp")
                    nc.vector.tensor_tensor(tmp2[:], ps_a[:], t2s[:, tcol], mult)
                    tmp1 = get_tmp1()
                    # NOT gpsimd: mixing tensor_tensor with partition_broadcast
                    # on GpSimd thrashes the Q7 library (~6us reload per switch)
                    nc.vector.tensor_tensor(slabs[s][:, tcol], tmp1[:], tmp2[:], add)
                return go, 1730

            def chain_v(tcn, tb):
                def go():
                    xtc = xtcs[tcn]
                    kb = 4 * tcn + tb
                    psv = psW.tile([128, 256], F32, name="psv", tag="w")
                    for cb in range(N_CB):
                        nc.tensor.matmul(
                            psv[:],
                            lhsT=xtc[:, QCHUNK * cb + 128 * tb : QCHUNK * cb + 128 * (tb + 1)],
                            rhs=wvs[:, 256 * cb : 256 * (cb + 1)],
                            start=(cb == 0), stop=(cb == N_CB - 1),
                        )
                    # Scalar engine is idle during phase A; PSUM->SBUF copy is
                    # cheap there and keeps the Vector queue free for rope math
                    nc.scalar.copy(
                        vs4[:, :, kb, 0:64], psv[:].rearrange("p (h d) -> p h d", h=4)
                    )
                return go, 880

            def make_a_chains(tcn):
                chains = []
                if tcn + 1 < NQC_:
                    chains.append((tcn, chain_prefetch(tcn + 1)))
                for s in range(4):
                    holder = {}
                    m_go, m_ns = chain_m(tcn, s)

                    def m_wrap(m_go=m_go, holder=holder):
                        holder["tmp1"] = m_go()
                    chains.append((tcn, (m_wrap, m_ns)))
                    a_go, a_ns = chain_a(tcn, s, lambda holder=holder: holder["tmp1"])
                    chains.append((tcn, (a_go, a_ns)))
                for tb in range(4):
                    chains.append((tcn, chain_v(tcn, tb)))
                return chains

            awork = deque()
            for tcn in range(1, NQC_):
                awork.extend(make_a_chains(tcn))

            # phase A for tcn=0: dense
            for _, (go, _ns) in make_a_chains(0):
                go()

            state = {"deficit": 0.0}

            def pace(max_tcn, budget=True):
                while awork and awork[0][0] <= max_tcn:
                    _, (go, cost) = awork[0]
                    if budget and state["deficit"] < cost:
                        break
                    awork.popleft()
                    go()
                    state["deficit"] -= cost

            # ---------------- attention + proj + RS ----------------
            ROUND_GAP = 280.0
            pending_tail = []  # deferred softmax tails

            def emit_tails():
                while pending_tail:
                    yps, hp, u, qcol, r = pending_tail.pop(0)
                    # stage denom at partition 0 (custom DVE op requires it)
                    den = pb2.tile([1, QCHUNK], F32, name="den", tag="den")
                    nc.vector.tensor_copy(den[:], yps[64:65, :])
                    recip = pb2.tile([1, QCHUNK], F32, name="recip", tag="recip")
                    nc.vector.reciprocal_approx_fast(out=recip[:], in_=den[:])
                    bcast = pb2.tile([64, QCHUNK], F32, name="bcast", tag="bcast")
                    nc.gpsimd.partition_broadcast(bcast[:], recip[:])
                    nc.vector.tensor_tensor(
                        yslabs[hp][64 * u : 64 * u + 64, qcol],
                        yps[0:64, :], bcast[:], mult,
                    )
                    if DBG:
                        nc.sync.dma_start(out=dbg_recip[r : r + 1, :], in_=recip[:])

            for qc in range(NQC_):
                qcol = slice(QCHUNK * qc, QCHUNK * (qc + 1))
                nblocks = 4 * qc + 4
                for j in range(4):
                    hp, u = j // 2, j % 2
                    qsl = slabs[hp]
                    ksl = slabs[2 + hp]
                    off = 64 * u
                    yps = psY.tile([65, QCHUNK], F32, name="yps", tag="yps")
                    ets = {}
                    for pr in range(nblocks // 2):
                        sp = psS.tile([128, 1024], F32, name="sp", tag="sp")
                        for w in range(2):
                            kb = 2 * pr + w
                            nc.tensor.matmul(
                                sp[:, 512 * w : 512 * (w + 1)],
                                lhsT=ksl[off : off + 64, 128 * kb : 128 * (kb + 1)],
                                rhs=qsl[off : off + 64, qcol],
                                start=True, stop=True,
                            )
                        et = pb.tile([128, 1024], BF16, name="et", tag="et")
                        nc.scalar.activation(
                            et[:], sp[:], mybir.ActivationFunctionType.Exp,
                            scale=0.125,
                        )
                        if pr >= 2 * qc:  # pair straddles the causal diagonal
                            jd0 = 2 * (pr - 2 * qc)
                            nc.vector.tensor_tensor(
                                et[:], et[:],
                                mks[:, 512 * jd0 : 512 * jd0 + 1024], mult,
                            )
                        ets[pr] = et
                        if pr == 1:
                            emit_tails()
                        # interleave ready phase-A work into the exp-bound gap
                        state["deficit"] += ROUND_GAP * 4
                        pace(qc + 1)
                        # A@V one pr behind the scores
                        if pr >= 1:
                            prev = pr - 1
                            etp = ets.pop(prev)
                            for w in range(2):
                                kb = 2 * prev + w
                                nc.tensor.matmul(
                                    yps[:],
                                    lhsT=vslab[:, (j * NKB_ + kb) * 65 : (j * NKB_ + kb + 1) * 65],
                                    rhs=etp[:, 512 * w : 512 * (w + 1)],
                                    start=(kb == 0), stop=False,
                                )
                    prev = nblocks // 2 - 1
                    etp = ets.pop(prev)
                    for w in range(2):
                        kb = 2 * prev + w
                        nc.tensor.matmul(
                            yps[:],
                            lhsT=vslab[:, (j * NKB_ + kb) * 65 : (j * NKB_ + kb + 1) * 65],
                            rhs=etp[:, 512 * w : 512 * (w + 1)],
                            start=(kb == 0), stop=(kb == nblocks - 1),
                        )
                    pending_tail.append((yps, hp, u, qcol, 4 * qc + j))

                # drain remaining A-work for the next T-chunk, tails overlap it
                state["deficit"] += 2 * 1730
                pace(qc + 1)
                emit_tails()
                pace(qc + 1, budget=False)

                # proj partials for this T-chunk, then ReduceScatter
                for e in range(8):
                    pso = psW.tile([128, QCHUNK], F32, name="pso", tag="w")
                    for cb in range(2):
                        nc.tensor.matmul(
                            pso[:],
                            lhsT=wps[:, C * cb + 128 * e : C * cb + 128 * (e + 1)],
                            rhs=yslabs[cb][:, qcol],
                            start=(cb == 0), stop=(cb == 1),
                        )
                    osb = pc_o.tile([128, QCHUNK], BF16, name="osb", tag="osb")
                    nc.vector.tensor_copy(osb[:], pso[:])
                    nc.sync.dma_start(out=rsin[qc][128 * e : 128 * (e + 1), :], in_=osb[:])
                nc.gpsimd.collective_compute(
                    "ReduceScatter", add, replica_groups=RG,
                    ins=[rsin[qc][:].opt()], outs=[rsout[qc][:].opt()],
                )

            # output hops: DRAM->DRAM on the Scalar DMA queue. tile_wait_until
            # pins them to the END of the scalar queue in the scheduler's model
            # (they wait on RS completion; scheduled early they head-block the
            # engine queue and freeze the exp stream for tens of us).
            for qc in range(NQC_):
                qcol = slice(QCHUNK * qc, QCHUNK * (qc + 1))
                with tc.tile_wait_until(1.0 + 0.1 * qc):
                    nc.scalar.dma_start(out=out[:, qcol], in_=rsout[qc][:, :])
            if DBG:
                nc.sync.dma_start(out=dbg_slab, in_=slabs[0][:])
                nc.sync.dma_start(out=dbg_vsl, in_=vslab[:])
                nc.sync.dma_start(out=dbg_ysl, in_=yslabs[0][:])
                nc.sync.dma_start(out=dbg_rso, in_=rsout[0][:, :])

    nc.compile()
    return nc


_NC_CACHE = {}


def get_nc():
    if "nc" not in _NC_CACHE:
        _NC_CACHE["nc"] = build_nc()
    return _NC_CACHE["nc"]


def assemble(results):
    out = np.empty((B, T, C), dtype=np.float32)
    for core in range(N_CORES):
        b, r = core // 4, core % 4
        out[b, :, 256 * r : 256 * (r + 1)] = np.asarray(results[core]["out"], dtype=np.float32).T
    return out


def kernel(x, w_attn, w_proj):
    in_maps = prepare_in_maps(x, w_attn, w_proj)
    nc = get_nc()
    res = run_bass_kernel_spmd(nc, in_maps, core_ids=list(range(N_CORES)))
    return assemble(res.results)
